# revision 32
# baseline (speedup 1.0000x reference)
"""Self-contained Trainium2 Bass kernel for nn_CalculateFlow.

Block-matching optical flow: binomial blur + u8 quantize, 7x7 SAD search with
5x5 templates (spiral tie-break argmin), Lucas-Kanade subpixel refinement on
the template ring, 3x3 median filter. Sharded row-wise across 8 NeuronCores
(68 rows/core + halos), fully data-parallel.

kernel(f_img, g_img) takes the full [1,1,544,960] fp32 inputs and returns the
full [1,2,544,960] fp32 flow.
"""
import numpy as np
import concourse.bass as bass
import concourse.bacc as bacc
import concourse.mybir as mybir
from concourse.tile import TileContext
from concourse import bass_utils

H, W = 544, 960
RPC = 68
W2 = 976
X0 = 8
AluOp = mybir.AluOpType
ActFn = mybir.ActivationFunctionType
f32d = mybir.dt.float32
f32r = mybir.dt.float32r
f16d = mybir.dt.float16
i16d = mybir.dt.int16
i32d = mybir.dt.int32

LO = X0 - 2          # AD-domain col 0 (= image x -2); even
AW = W + 4           # 964 AD-domain width


def spiral_rank():
    sr = 3
    s = 2 * sr + 1
    rank = np.zeros((s, s), np.int32)
    order = [(0, 0)]
    y = x = 0
    step, d = 1, 0
    dirs = [(0, 1), (1, 0), (0, -1), (-1, 0)]
    while len(order) < s * s:
        for _ in range(2):
            dy, dx = dirs[d % 4]
            for _ in range(step):
                y += dy; x += dx
                if abs(y) <= sr and abs(x) <= sr:
                    order.append((y, x))
            d += 1
        step += 1
    for r, (yy, xx) in enumerate(order):
        rank[yy + sr, xx + sr] = r
    return rank


def host_inputs(c, f_img, g_img):
    """Build the per-core input dict (numpy), f_img/g_img: [H, W] fp32."""
    r0 = c * RPC

    def hslice(img, top_halo, nrows):
        rows = [img[min(max(r0 - top_halo + p, 0), H - 1)] for p in range(nrows)]
        out = np.zeros((nrows, W2), np.float32)
        out[:, X0:X0 + W] = np.stack(rows)
        return out

    def rowmask(top_halo, nrows):
        r = np.arange(nrows) + r0 - top_halo
        return ((r >= 0) & (r < H)).astype(np.float32)[:, None]

    gradband = np.zeros((80, 74), np.float32)
    for p in range(74):
        r = r0 - 3 + p
        if r < 0 or r >= H:
            continue
        rm = min(max(r - 1, 0), H - 1)
        rp = min(max(r + 1, 0), H - 1)
        gradband[rp - (r0 - 6), p] += np.float32(1.0 / 32.0)
        gradband[rm - (r0 - 6), p] -= np.float32(1.0 / 32.0)

    band5 = np.zeros((74, 70), np.float32)
    band2 = np.zeros((74, 70), np.float32)
    for m in range(70):
        band5[m:m + 5, m] = 1.0
        band2[m, m] = 1.0
        band2[m + 4, m] = 1.0

    medup = np.zeros((70, 70), np.float32)
    meddn = np.zeros((70, 70), np.float32)
    for p in range(70):
        r = r0 - 1 + p
        rm = min(max(r - 1, 0), H - 1) - (r0 - 1)
        rp = min(max(r + 1, 0), H - 1) - (r0 - 1)
        rm = min(max(rm, 0), 69)
        rp = min(max(rp, 0), 69)
        medup[rm, p] = 1.0
        meddn[rp, p] = 1.0

    return {
        "f_t": hslice(f_img, 7, 82),
        "g_t": hslice(g_img, 4, 76),
        "fmask": rowmask(6, 80),
        "m74": rowmask(3, 74),
        "gradband": gradband,
        "band5h": band5.astype(np.float16),
        "band2h": band2.astype(np.float16),
        "band5f": band5,
        "band2f": band2,
        "medup": medup,
        "meddn": meddn,
    }


INPUT_SPECS = [
    ("f_t", [82, W2], f32d),
    ("g_t", [76, W2], f32d),
    ("fmask", [80, 1], f32d),
    ("m74", [74, 1], f32d),
    ("gradband", [80, 74], f32d),
    ("band5h", [74, 70], f16d),
    ("band2h", [74, 70], f16d),
    ("band5f", [74, 70], f32r),
    ("band2f", [74, 70], f32r),
    ("medup", [70, 70], f32r),
    ("meddn", [70, 70], f32r),
]


def build_kernel(tc, out_ap, in_aps, dbg=None):
    """Emit the full per-core program. in_aps: dict name->AP (DRAM)."""
    nc = tc.nc
    rank = spiral_rank()
    RNG = range

    # fp32 matmuls run at 4 cyc/row (emitted as 2 half-speed passes); float32r
    # with free dim >= 256 runs at 1 cyc/row. Bitcast all fp32 PE operands.
    def r32(ap):
        return ap.bitcast(mybir.dt.float32r)

    import contextlib
    stack = contextlib.ExitStack()
    pool = stack.enter_context(tc.tile_pool(name="main", bufs=1))
    tpool = stack.enter_context(tc.tile_pool(name="tmp", bufs=1))
    ppool = stack.enter_context(tc.tile_pool(name="ps", bufs=1, space="PSUM"))
    rpool = stack.enter_context(tc.tile_pool(name="psr", bufs=1, space="PSUM"))

    def dload(name, shape, dtype):
        t = pool.tile(shape, dtype, tag=name, name=name)
        nc.sync.dma_start(t[:, :], in_aps[name])
        return t

    f_t = dload("f_t", [82, W2], f32d)
    g_t = dload("g_t", [76, W2], f32d)
    fmask = dload("fmask", [80, 1], f32d)
    m74 = dload("m74", [74, 1], f32d)
    gradband = dload("gradband", [80, 74], f32d)
    band5h = dload("band5h", [74, 70], f16d)
    band2h = dload("band2h", [74, 70], f16d)
    band5f = dload("band5f", [74, 70], f32r)
    band2f = dload("band2f", [74, 70], f32r)
    medup = dload("medup", [70, 70], f32r)
    meddn = dload("meddn", [70, 70], f32r)

    # ---------------- blur f (deferred /16) ----------------
    # shifted row copies via DMA
    fsh1 = pool.tile([80, W2], f32d, tag="wA")
    fsh2 = pool.tile([80, W2], f32d, tag="wB")
    nc.sync.dma_start(fsh1[:, :], f_t[1:81, :])
    nc.sync.dma_start(fsh2[:, :], f_t[2:82, :])
    fv = pool.tile([80, W2], f32d, tag="wC")
    # fv = (fsh1*2 + f_t[0:80]) + fsh2   (matches ref order (top + 2x) + bot)
    nc.vector.tensor_scalar_mul(fv[:, :], fsh1[:, :], 2.0)
    nc.vector.tensor_add(fv[:, :], fv[:, :], f_t[0:80, :])
    nc.vector.tensor_add(fv[:, :], fv[:, :], fsh2[:, :])
    # x-edge replicate then h-blur
    nc.vector.tensor_copy(fv[:, X0 - 1:X0], fv[:, X0:X0 + 1])
    nc.vector.tensor_copy(fv[:, X0 + W:X0 + W + 1], fv[:, X0 + W - 1:X0 + W])
    blur2f = pool.tile([80, W2], f32d, tag="wD")
    nc.vector.memset(blur2f[:, :], 0.0)
    # blur2f[x] = (fv[x]*2 + fv[x-1]) + fv[x+1] over x in [X0-1 .. X0+W+1)? need
    # [X0-1, X0+W+1) for gradient x-shifts; compute on [X0-2..X0+W+2) safe width 964+?
    cA, cB = X0 - 2, X0 + W + 2   # compute region (zeros outside image propagate fine)
    nc.vector.scalar_tensor_tensor(out=blur2f[:, cA:cB], in0=fv[:, cA:cB], scalar=2.0,
                                   in1=fv[:, cA - 1:cB - 1], op0=AluOp.mult, op1=AluOp.add)
    nc.vector.tensor_add(blur2f[:, cA:cB], blur2f[:, cA:cB], fv[:, cA + 1:cB + 1])
    # replicate blur cols at image x-edges (for gradient), matching reference's
    # edge-pad of f_blur itself
    nc.vector.tensor_copy(blur2f[:, X0 - 1:X0], blur2f[:, X0:X0 + 1])
    nc.vector.tensor_copy(blur2f[:, X0 + W:X0 + W + 1], blur2f[:, X0 + W - 1:X0 + W])

    # quantize f: round-half-even via fp32 magic-number trick, then mask -> fp16
    fxs = pool.tile([80, W2], f32d, tag="wF2")
    nc.vector.tensor_scalar(out=fxs[:, :], in0=blur2f[:, :], scalar1=float(255.0 / 16.0),
                            scalar2=8388608.0, op0=AluOp.mult, op1=AluOp.add)
    f16 = pool.tile([80, W2], f16d, tag="f16")
    nc.vector.tensor_scalar(out=f16[:, :], in0=fxs[:, :], scalar1=-8388608.0,
                            scalar2=fmask[:, 0:1], op0=AluOp.add, op1=AluOp.mult)
    nc.vector.memset(f16[:, 0:X0], 0.0)
    nc.vector.memset(f16[:, X0 + W:W2], 0.0)

    # ---------------- blur g ----------------
    gsh1 = pool.tile([74, W2], f32d, tag="wA")
    gsh2 = pool.tile([74, W2], f32d, tag="wB")
    nc.sync.dma_start(gsh1[:, :], g_t[1:75, :])
    nc.sync.dma_start(gsh2[:, :], g_t[2:76, :])
    gv = pool.tile([74, W2], f32d, tag="wC")
    nc.vector.tensor_scalar_mul(gv[:, :], gsh1[:, :], 2.0)
    nc.vector.tensor_add(gv[:, :], gv[:, :], g_t[0:74, :])
    nc.vector.tensor_add(gv[:, :], gv[:, :], gsh2[:, :])
    nc.vector.tensor_copy(gv[:, X0 - 1:X0], gv[:, X0:X0 + 1])
    nc.vector.tensor_copy(gv[:, X0 + W:X0 + W + 1], gv[:, X0 + W - 1:X0 + W])
    gb2 = pool.tile([74, W2], f32d, tag="wD")
    nc.vector.memset(gb2[:, :], 0.0)
    nc.vector.scalar_tensor_tensor(out=gb2[:, cA:cB], in0=gv[:, cA:cB], scalar=2.0,
                                   in1=gv[:, cA - 1:cB - 1], op0=AluOp.mult, op1=AluOp.add)
    nc.vector.tensor_add(gb2[:, cA:cB], gb2[:, cA:cB], gv[:, cA + 1:cB + 1])
    gxs = pool.tile([74, W2], f32d, tag="wF2")
    nc.vector.tensor_scalar(out=gxs[:, :], in0=gb2[:, :], scalar1=float(255.0 / 16.0),
                            scalar2=8388608.0, op0=AluOp.mult, op1=AluOp.add)
    g16 = pool.tile([74, W2], f16d, tag="g16")
    nc.vector.tensor_scalar(out=g16[:, :], in0=gxs[:, :], scalar1=-8388608.0,
                            scalar2=m74[:, 0:1], op0=AluOp.add, op1=AluOp.mult)
    nc.vector.memset(g16[:, 0:X0], 0.0)
    nc.vector.memset(g16[:, X0 + W:W2], 0.0)

    # ---------------- gradients ----------------
    # dfy via PE: gradband^T @ blur2f  -> [74, W2] (2 banks of 488)
    dfy = pool.tile([74, W2], f32d, tag="dfy")
    psg = rpool.tile([74, 2, 512], f32d, tag="rc0")
    nc.tensor.matmul(psg[:, 0, 0:488], gradband[:, :], blur2f[:, 0:488], start=True, stop=True)
    nc.tensor.matmul(psg[:, 1, 0:488], gradband[:, :], blur2f[:, 488:976], start=True, stop=True)
    nc.scalar.copy(dfy[:, :], psg[:, :, 0:488])
    nc.vector.memset(dfy[:, 0:X0], 0.0)
    nc.vector.memset(dfy[:, X0 + W:W2], 0.0)
    # dfx via shifts on blur2f rows 3..77 (f_blur rows r0-3..r0+70) * (1/32), masked
    dfx = pool.tile([74, W2], f32d, tag="dfx")
    b2c = pool.tile([74, W2], f32d, tag="wE")
    nc.sync.dma_start(b2c[:, :], blur2f[3:77, :])
    nc.vector.memset(dfx[:, :], 0.0)
    dsub = pool.tile([74, W], f32d, tag="wF")
    nc.vector.tensor_sub(dsub[:, :], b2c[:, X0 + 1:X0 + W + 1], b2c[:, X0 - 1:X0 + W - 1])
    nc.vector.tensor_scalar(out=dfx[:, X0:X0 + W], in0=dsub[:, :], scalar1=m74[:, 0:1],
                            scalar2=float(1.0 / 32.0), op0=AluOp.mult, op1=AluOp.mult)
    # apply m74 mask to dfy too (rows outside image -> 0)
    nc.vector.tensor_scalar(out=dfy[:, :], in0=dfy[:, :], scalar1=m74[:, 0:1],
                            scalar2=None, op0=AluOp.mult)
    # fp16 copies for the in-loop LK products
    dfx16 = pool.tile([74, W2], f16d, tag="dfx16", name="dfx16")
    dfy16 = pool.tile([74, W2], f16d, tag="dfy16", name="dfy16")
    nc.scalar.copy(dfx16[:, :], dfx[:, :])
    nc.scalar.copy(dfy16[:, :], dfy[:, :])

    if dbg is not None:
        for key, tile, sl in [("f16", f16, None), ("g16", g16, None),
                              ("dfx", dfx, None), ("dfy", dfy, None)]:
            if key in dbg:
                nc.gpsimd.dma_start(dbg[key], tile[:, :] if sl is None else sl)

    # ---------------- f_dj shifted copies (even and odd col parity) ----------------
    fdj_e = []
    for dj in RNG(-3, 4):
        te = pool.tile([74, W2], f16d, tag=f"fdj_e{dj}", name=f"fdj_e{dj}")
        nc.sync.dma_start(te[:, :], f16[3 + dj:77 + dj, :])
        fdj_e.append(te)

    # ---------------- LK fixed-part products (independent of the sweep) ----
    prodA = tpool.tile([74, AW], f32r, tag="prodA")
    nc.scalar.activation(prodA[:, :], dfx[:, LO:LO + AW], ActFn.Square)
    prodC = tpool.tile([74, AW], f32r, tag="prodC")
    nc.scalar.activation(prodC[:, :], dfy[:, LO:LO + AW], ActFn.Square)
    prodB = tpool.tile([74, AW], f32r, tag="prodB")
    nc.gpsimd.tensor_tensor(prodB[:, :], dfx[:, LO:LO + AW], dfy[:, LO:LO + AW],
                            AluOp.mult)
    a_rs = pool.tile([70, W], f32d, tag="rs_a", name="rs_a")
    b_rs = pool.tile([70, W], f32d, tag="rs_b", name="rs_b")
    d_rs = pool.tile([70, W], f32d, tag="rs_d", name="rs_d")
    det = pool.tile([70, W], f32d, tag="det")
    rdet = tpool.tile([70, W], f32d, tag="rdet")
    valid = tpool.tile([70, W], f32d, tag="valid")

    # ---------------- sweep ----------------
    m = pool.tile([70, W], f32d, tag="m")
    nc.vector.memset(m[:, :], 3.0e7)
    T2x = pool.tile([70, 2, 480], f32d, tag="T2x")
    T2y = pool.tile([70, 2, 480], f32d, tag="T2y")
    nc.vector.memset(T2x[:, :, :], 0.0)
    nc.vector.memset(T2y[:, :, :], 0.0)

    KORD = (0, 4, 1, 2, 3)   # band5, band5, band2 x3 -> fewer weight switches
    # Software-pipelined sweep: iteration i emits its h-chain/products and ring
    # matmuls, while the m/mask/pred consumption of iteration i-1 is interleaved
    # so DVE never waits on same-iteration PE results.
    pend = None  # (bias, cps, mask_tile, rcx, rcy) of previous iteration

    def flush_pend():
        nonlocal pend
        if pend is None:
            return
        bias_p, cps_p, rcx_p, rcy_p = pend
        nc.vector.scalar_tensor_tensor(out=m[:, :], in0=cps_p[:, :, 0:480], scalar=bias_p,
                                       in1=m[:, :], op0=AluOp.add, op1=AluOp.min)
        mask = tpool.tile([70, 2, 480], i16d, tag="mask", bufs=2)
        nc.vector.scalar_tensor_tensor(out=mask[:, :, :], in0=cps_p[:, :, 0:480], scalar=bias_p,
                                       in1=m[:, :], op0=AluOp.add, op1=AluOp.is_equal)
        nc.vector.copy_predicated(T2x[:, :, :], mask[:, :, :], rcx_p[:, :, 0:480])
        nc.vector.copy_predicated(T2y[:, :, :], mask[:, :, :], rcy_p[:, :, 0:480])
        pend = None

    for dj in RNG(-3, 4):
        for di in RNG(-3, 4):
            r_s = int(rank[dj + 3, di + 3])
            bias = float(np.float32(r_s / 64.0 + (dj + 3) / 512.0 + (di + 3) / 4096.0))
            fs = fdj_e[dj + 3]
            base = LO + di
            # diff (fp16)
            diff = tpool.tile([74, AW], f16d, tag="diff", bufs=3)
            nc.vector.tensor_sub(diff[:, :], fs[:, base:base + AW], g16[:, LO:LO + AW])
            ad = tpool.tile([74, AW], f16d, tag="ad", bufs=3)
            nc.scalar.activation(ad[:, :], diff[:, :], ActFn.Abs)
            # hbox: st2 st4 h5
            st2 = tpool.tile([74, AW - 2], f16d, tag="st2", bufs=2)
            nc.vector.tensor_add(st2[:, 0:AW - 2], ad[:, 0:AW - 2], ad[:, 1:AW - 1])
            st4 = tpool.tile([74, W], f16d, tag="st4", bufs=2)
            nc.vector.tensor_add(st4[:, :], st2[:, 0:W], st2[:, 2:W + 2])
            h5 = tpool.tile([74, W], f16d, tag="h5", bufs=3)
            nc.vector.tensor_add(h5[:, :], st4[:, :], ad[:, 4:W + 4])
            # LK products on the SAD diff (fp16): rs(diff*df) = -255*rs(z*df)
            px = tpool.tile([74, AW], f16d, tag="px", bufs=3)
            nc.vector.tensor_mul(px[:, :], diff[:, :], dfx16[:, LO:LO + AW])
            py = tpool.tile([74, AW], f16d, tag="py", bufs=3)
            nc.gpsimd.tensor_tensor(py[:, :], diff[:, :], dfy16[:, LO:LO + AW],
                                    AluOp.mult)
            # ring sums first, cost last (PE order) so cps is consumed with slack
            it = (dj + 3) * 7 + (di + 3)
            rcx = rpool.tile([70, 2, 512], f32d, tag=f"rc{(2 * it) % 3}")
            rcy = rpool.tile([70, 2, 512], f32d, tag=f"rc{(2 * it + 1) % 3}")
            for half, (o0, o1) in enumerate(((0, 480), (480, 960))):
                for k in KORD:
                    bnd = band5h if k in (0, 4) else band2h
                    nc.tensor.matmul(rcx[:, half, 0:480], bnd[:, :], px[:, o0 + k:o1 + k],
                                     start=(k == 0), stop=(k == 3))
                for k in KORD:
                    bnd = band5h if k in (0, 4) else band2h
                    nc.tensor.matmul(rcy[:, half, 0:480], bnd[:, :], py[:, o0 + k:o1 + k],
                                     start=(k == 0), stop=(k == 3))
            # previous iteration's m/mask/preds now that our DVE work is queued
            flush_pend()
            cps = ppool.tile([70, 2, 512], f32d, tag="cps")
            nc.tensor.matmul(cps[:, 0, 0:480], band5h[:, :], h5[:, 0:480], start=True, stop=True)
            nc.tensor.matmul(cps[:, 1, 0:480], band5h[:, :], h5[:, 480:960], start=True, stop=True)
            pend = (bias, cps, rcx, rcy)
    flush_pend()

    if dbg is not None and "m" in dbg:
        nc.sync.dma_start(dbg["m"], m[:, :])
    if dbg is not None and "T2x" in dbg:
        nc.sync.dma_start(dbg["T2x"], T2x[:, :])

    # ---------------- decode vec (pure fp32; exact) ----------------
    # n = m*4096 is an exact fp32 integer; di+3 = n mod 8; dj+3 = floor(n/8) mod 8.
    # floor(x/8) for integer x: RNE(x/8 - 0.4375) has no ties -> exact.
    MAGIC = 8388608.0
    nq = pool.tile([70, W], f32d, tag="wA")
    nc.vector.tensor_scalar_mul(nq[:, :], m[:, :], 4096.0)
    q8 = pool.tile([70, W], f32d, tag="wB")
    nc.vector.tensor_scalar(out=q8[:, :], in0=nq[:, :], scalar1=0.125, scalar2=-0.4375,
                            op0=AluOp.mult, op1=AluOp.add)
    nc.vector.tensor_scalar(out=q8[:, :], in0=q8[:, :], scalar1=MAGIC, scalar2=-MAGIC,
                            op0=AluOp.add, op1=AluOp.add)
    di3 = pool.tile([70, W], f32d, tag="wC")
    nc.vector.scalar_tensor_tensor(out=di3[:, :], in0=q8[:, :], scalar=-8.0,
                                   in1=nq[:, :], op0=AluOp.mult, op1=AluOp.add)
    vecx = pool.tile([70, W], f32d, tag="vecx")
    nc.vector.tensor_scalar(out=vecx[:, :], in0=di3[:, :], scalar1=-1.0, scalar2=3.0,
                            op0=AluOp.mult, op1=AluOp.add)
    q64 = pool.tile([70, W], f32d, tag="wD")
    nc.vector.tensor_scalar(out=q64[:, :], in0=q8[:, :], scalar1=0.125, scalar2=-0.4375,
                            op0=AluOp.mult, op1=AluOp.add)
    nc.vector.tensor_scalar(out=q64[:, :], in0=q64[:, :], scalar1=MAGIC, scalar2=-MAGIC,
                            op0=AluOp.add, op1=AluOp.add)
    dj3 = pool.tile([70, W], f32d, tag="wE")
    nc.vector.scalar_tensor_tensor(out=dj3[:, :], in0=q64[:, :], scalar=-8.0,
                                   in1=q8[:, :], op0=AluOp.mult, op1=AluOp.add)
    vecy = pool.tile([70, W], f32d, tag="vecy")
    nc.vector.tensor_scalar(out=vecy[:, :], in0=dj3[:, :], scalar1=-1.0, scalar2=3.0,
                            op0=AluOp.mult, op1=AluOp.add)

    # ---------------- LK fixed-part ring sums + det ----------------
    for prod_h, rs_h, snm in ((prodA, a_rs, 0), (prodB, b_rs, 1), (prodC, d_rs, 2)):
        slot = rpool.tile([70, 2, 512], f32d, tag=f"rc{snm}", name=f"rsum{snm}")
        for half, (o0, o1) in enumerate(((0, 480), (480, 960))):
            for k in KORD:
                bnd = band5f if k in (0, 4) else band2f
                nc.tensor.matmul(slot[:, half, 0:480], bnd[:, :],
                                 prod_h[:, o0 + k:o1 + k],
                                 start=(k == 0), stop=(k == 3))
        nc.scalar.copy(rs_h[:, :], slot[:, :, 0:480])
    bsq = tpool.tile([70, W], f32d, tag="tA")
    nc.scalar.activation(bsq[:, :], b_rs[:, :], ActFn.Square)
    nc.vector.tensor_mul(det[:, :], a_rs[:, :], d_rs[:, :])
    nc.vector.tensor_sub(det[:, :], det[:, :], bsq[:, :])
    safe = tpool.tile([70, W], f32d, tag="tB")
    nc.vector.tensor_scalar_max(safe[:, :], det[:, :], 1e-7)
    nc.vector.reciprocal(rdet[:, :], safe[:, :])
    nc.vector.tensor_scalar(out=valid[:, :], in0=det[:, :], scalar1=1e-7,
                            scalar2=None, op0=AluOp.is_gt)

    # ---------------- LK solve ----------------
    # p = rs(z*dfx) with z = (g - f_sel)/255 and T2 = rs((f_sel - g)*dfx)
    ninv255 = float(np.float32(-1.0 / 255.0))
    p_ = pool.tile([70, W], f32d, tag="p_")
    nc.vector.tensor_scalar_mul(p_[:, :], T2x[:, :, :], ninv255)
    q_ = pool.tile([70, W], f32d, tag="q_")
    nc.vector.tensor_scalar_mul(q_[:, :], T2y[:, :, :], ninv255)

    def subcomp(c1, t1, c2, t2, name):
        # (c1*t1 - c2*t2) * rdet, gated
        u = tpool.tile([70, W], f32d, tag="tU", name=f"u_{name}")
        nc.vector.tensor_mul(u[:, :], c1[:, :], t1[:, :])
        v = tpool.tile([70, W], f32d, tag="tA", name=f"v_{name}")
        nc.vector.tensor_mul(v[:, :], c2[:, :], t2[:, :])
        nc.vector.tensor_sub(u[:, :], u[:, :], v[:, :])
        nc.vector.tensor_mul(u[:, :], u[:, :], rdet[:, :])
        # gate: |u| < 1 and valid  (via u^2 < 1; ScalarE Square)
        usq = tpool.tile([70, W], f32d, tag="lo3", name=f"usq_{name}")
        nc.scalar.activation(usq[:, :], u[:, :], ActFn.Square)
        au = tpool.tile([70, W], f32d, tag="tB", name=f"au_{name}")
        nc.vector.tensor_scalar(out=au[:, :], in0=usq[:, :], scalar1=1.0, scalar2=None,
                                op0=AluOp.is_lt)
        nc.vector.tensor_mul(au[:, :], au[:, :], valid[:, :])
        nc.vector.tensor_mul(u[:, :], u[:, :], au[:, :])
        return u

    sub_u = subcomp(d_rs, p_, b_rs, q_, "su")
    sub_v = subcomp(a_rs, q_, b_rs, p_, "sv")

    flow_u = pool.tile([70, W + 2], f32r, tag="flow_u")
    flow_v = pool.tile([70, W + 2], f32r, tag="flow_v")
    nc.vector.tensor_add(flow_u[:, 1:W + 1], vecx[:, :], sub_u[:, :])
    nc.vector.tensor_add(flow_v[:, 1:W + 1], vecy[:, :], sub_v[:, :])
    # x-edge replicate
    for fl in (flow_u, flow_v):
        nc.vector.tensor_copy(fl[:, 0:1], fl[:, 1:2])
        nc.vector.tensor_copy(fl[:, W + 1:W + 2], fl[:, W:W + 1])

    if dbg is not None and "flow_v" in dbg:
        nc.sync.dma_start(dbg["flow_v"], flow_v[:, :])

    # ---------------- median ----------------
    def median(fl, name, out_slice):
        # row shifts via PE bands (fp32r, exact single-coeff rows)
        WP = W + 2
        pu = rpool.tile([70, 2, 512], f32d, tag="rc0", name=f"pu_{name}")
        nc.tensor.matmul(pu[:, 0, 0:482], medup[:, :], fl[:, 0:482], start=True, stop=True)
        nc.tensor.matmul(pu[:, 1, 0:480], medup[:, :], fl[:, 482:WP], start=True, stop=True)
        up = pool.tile([70, WP], f32d, tag="wA", name=f"up_{name}")
        nc.scalar.copy(up[:, 0:482], pu[:, 0, 0:482])
        nc.scalar.copy(up[:, 482:WP], pu[:, 1, 0:480])
        pd = rpool.tile([70, 2, 512], f32d, tag="rc1", name=f"pd_{name}")
        nc.tensor.matmul(pd[:, 0, 0:482], meddn[:, :], fl[:, 0:482], start=True, stop=True)
        nc.tensor.matmul(pd[:, 1, 0:480], meddn[:, :], fl[:, 482:WP], start=True, stop=True)
        dn = pool.tile([70, WP], f32d, tag="wB", name=f"dn_{name}")
        nc.scalar.copy(dn[:, 0:482], pd[:, 0, 0:482])
        nc.scalar.copy(dn[:, 482:WP], pd[:, 1, 0:480])
        A, B, C = up, fl, dn
        lo3 = tpool.tile([70, WP], f32d, tag="lo3")
        hi3 = tpool.tile([70, WP], f32d, tag="hi3")
        md3 = tpool.tile([70, WP], f32d, tag="md3")
        tmn = tpool.tile([70, WP], f32d, tag="tmn")
        nc.vector.tensor_tensor(tmn[:, :], A[:, :], B[:, :], AluOp.min)
        nc.vector.tensor_tensor(hi3[:, :], A[:, :], B[:, :], AluOp.max)
        nc.vector.tensor_tensor(lo3[:, :], tmn[:, :], C[:, :], AluOp.min)
        # md3 = max(min(hi3, C), tmn)
        nc.vector.tensor_tensor(md3[:, :], hi3[:, :], C[:, :], AluOp.min)
        nc.vector.tensor_tensor(md3[:, :], md3[:, :], tmn[:, :], AluOp.max)
        nc.vector.tensor_tensor(hi3[:, :], hi3[:, :], C[:, :], AluOp.max)
        # horizontal: mx = min3(hi3), mn = max3(lo3), mdm = med3(md3)
        mx = tpool.tile([70, W], f32d, tag="tU")
        nc.vector.tensor_tensor(mx[:, :], hi3[:, 0:W], hi3[:, 1:W + 1], AluOp.min)
        nc.vector.tensor_tensor(mx[:, :], mx[:, :], hi3[:, 2:W + 2], AluOp.min)
        mn = tpool.tile([70, W], f32d, tag="tA")
        nc.vector.tensor_tensor(mn[:, :], lo3[:, 0:W], lo3[:, 1:W + 1], AluOp.max)
        nc.vector.tensor_tensor(mn[:, :], mn[:, :], lo3[:, 2:W + 2], AluOp.max)
        m2n = tpool.tile([70, W], f32d, tag="tB")
        m2x = tpool.tile([70, W], f32d, tag="rdet")
        nc.vector.tensor_tensor(m2n[:, :], md3[:, 0:W], md3[:, 1:W + 1], AluOp.min)
        nc.vector.tensor_tensor(m2x[:, :], md3[:, 0:W], md3[:, 1:W + 1], AluOp.max)
        mdm = tpool.tile([70, W], f32d, tag="valid")
        nc.vector.tensor_tensor(mdm[:, :], m2x[:, :], md3[:, 2:W + 2], AluOp.min)
        nc.vector.tensor_tensor(mdm[:, :], mdm[:, :], m2n[:, :], AluOp.max)
        # final med3(mx, mdm, mn)
        f1 = tpool.tile([70, W], f32d, tag="f1")
        f2 = tpool.tile([70, W], f32d, tag="f2")
        nc.vector.tensor_tensor(f1[:, :], mx[:, :], mdm[:, :], AluOp.min)
        nc.vector.tensor_tensor(f2[:, :], mx[:, :], mdm[:, :], AluOp.max)
        nc.vector.tensor_tensor(f2[:, :], f2[:, :], mn[:, :], AluOp.min)
        nc.vector.tensor_tensor(f2[:, :], f2[:, :], f1[:, :], AluOp.max)
        nc.sync.dma_start(out_slice, f2[1:69, :])

    median(flow_v, "v", out_ap[0, :, :])
    median(flow_u, "u", out_ap[1, :, :])

    stack.close()


# ---------------------------------------------------------------------------
_CACHE = {}


def _get_runner(n_cores=8):
    """Build the Bass module once and return a cached jitted SPMD callable."""
    if "runner" in _CACHE:
        return _CACHE["runner"]
    import jax
    from jax.sharding import Mesh, PartitionSpec
    from jax.experimental.shard_map import shard_map
    from concourse import bass2jax
    import concourse.mybir as mybir_

    nc = bacc.Bacc("TRN2", num_devices=n_cores)
    in_aps = {}
    for name, shape, dtype in INPUT_SPECS:
        in_aps[name] = nc.dram_tensor(name, shape, dtype, kind="ExternalInput").ap()
    out_t = nc.dram_tensor("flow_out", [2, RPC, W], mybir.dt.float32,
                           kind="ExternalOutput")
    with TileContext(nc) as tc:
        build_kernel(tc, out_t.ap(), in_aps)
    nc.compile()

    bass2jax.install_neuronx_cc_hook()
    partition_name = nc.partition_id_tensor.name if nc.partition_id_tensor else None
    in_names, out_names, out_avals, zero_shapes = [], [], [], []
    for alloc in nc.m.functions[0].allocations:
        if not isinstance(alloc, mybir.MemoryLocationSet):
            continue
        name = alloc.memorylocations[0].name
        if alloc.kind == "ExternalInput":
            if name != partition_name:
                in_names.append(name)
        elif alloc.kind == "ExternalOutput":
            out_names.append(name)
            shape = tuple(alloc.tensor_shape)
            dtype = mybir.dt.np(alloc.dtype)
            out_avals.append(jax.core.ShapedArray(shape, dtype))
            zero_shapes.append((shape, dtype))
    n_params = len(in_names)
    all_names = list(in_names) + list(out_names)
    if partition_name is not None:
        all_names.append(partition_name)
    donate = tuple(range(n_params, n_params + len(out_names)))

    def _body(*args):
        operands = list(args)
        if partition_name is not None:
            operands.append(bass2jax.partition_id_tensor())
        outs = bass2jax._bass_exec_p.bind(
            *operands,
            out_avals=tuple(out_avals),
            in_names=tuple(all_names),
            out_names=tuple(out_names),
            lowering_input_output_aliases=(),
            sim_require_finite=True,
            sim_require_nnan=True,
            nc=nc,
        )
        return tuple(outs)

    devices = jax.devices()[:n_cores]
    mesh = Mesh(np.asarray(devices), ("core",))
    in_specs = (PartitionSpec("core"),) * (n_params + len(out_names))
    out_specs = (PartitionSpec("core"),) * len(out_names)
    sharded = jax.jit(
        shard_map(_body, mesh=mesh, in_specs=in_specs, out_specs=out_specs,
                  check_rep=False),
        donate_argnums=donate, keep_unused=True,
    )
    runner = {
        "fn": sharded, "in_names": in_names, "out_names": out_names,
        "zero_shapes": zero_shapes, "n_cores": n_cores,
    }
    _CACHE["runner"] = runner
    return runner


def _concat_inputs(runner, in_maps):
    n_cores = runner["n_cores"]
    return [
        np.concatenate([np.asarray(in_maps[c][nm]) for c in range(n_cores)], axis=0)
        for nm in runner["in_names"]
    ]


def _zero_outs(runner):
    n_cores = runner["n_cores"]
    return [np.zeros((n_cores * s[0], *s[1:]), d) for s, d in runner["zero_shapes"]]


def kernel(f_img, g_img):
    f_img = np.ascontiguousarray(np.asarray(f_img), dtype=np.float32)
    g_img = np.ascontiguousarray(np.asarray(g_img), dtype=np.float32)
    assert f_img.shape == (1, 1, H, W) and g_img.shape == (1, 1, H, W)
    runner = _get_runner(8)
    f2, g2 = f_img[0, 0], g_img[0, 0]
    in_maps = [host_inputs(c, f2, g2) for c in range(8)]
    concat_in = _concat_inputs(runner, in_maps)
    outs = runner["fn"](*concat_in, *_zero_outs(runner))
    flow = np.asarray(outs[0]).reshape(8, 2, RPC, W)
    out = np.concatenate([flow[c] for c in range(8)], axis=1)
    return out[None].astype(np.float32)

